# revision 17
# baseline (speedup 1.0000x reference)
"""Trainium2 Bass kernel for nn_PoolNU: gather + max-pool over neighbour table.

reference:
    x: (8, 128, 65536) f32, neighbours: (9, 16384) int
    out[b, c, j] = max_k x[b, c, neighbours[k, j]]

Strategy:
    - The neighbour table is shared across (b, c), so one gathered "row"
      carries ALL batches and channels for a location: x repacked host-side to
      (65536, B*C=1024). Values are cast to bf16 (tolerance is 2e-2; bf16
      rounding is ~0.2%), halving all HBM traffic vs f32.
    - Output locations (16384) are sharded across the 8 NeuronCores (2048
      per core).
    - The bottleneck is SWDGE descriptor generation on the GpSimd Q7 cores
      (~9.4ns/index, serialized).  To cut descriptor count, the host packs
      the per-core table in PAIR rows: for output j, slots (2f, 2f+1) are
      stored as adjacent 2KB rows, so one 4KB descriptor fetches two
      neighbour values.  Slot 8 stays as single 2KB rows.  Per output:
      4 pair descriptors + 1 single = 10240 descriptors/core instead of
      18432.  (8KB descriptors crash the ucode; 4KB is the proven max.)
    - Device: per 2-tile group one 1024-index dma_gather covering all 4 pair
      regions; per 8 tiles one 1024-index gather for slot 8; vector max-tree
      in bf16; store 2KB rows. dma_gather calls round-robin over 4 SWDGE
      queues (pipelines descriptor gen a bit via instruction run-ahead).
    - Host reassembles (core, loc, b, c) -> (b, c, loc) and casts to f32.
"""

import sys

sys.path.insert(0, "/opt/trn_rl_repo")

import ml_dtypes
import numpy as np

import concourse.mybir as mybir
from concourse import bacc, bass_utils
from concourse.tile import TileContext

B = 8
C = 128
LIN = 65536
K = 9
LOUT = 16384

P = 128
NCORE = 8
LPC = LOUT // NCORE          # locations per core (2048)
NTILE = LPC // P             # tiles per core (16)
E = B * C                    # elements per gathered row (1024)
NPAIR = 4                    # pair regions (slots 0-7)
UMAX = LPC * (2 * NPAIR + 1)  # table rows (18432)
NMAX = 1024                  # max indices per dma_gather call

BF16 = mybir.dt.bfloat16

NG = NTILE // 2              # 2-tile pair-gather groups (8)
NS = NTILE // 8              # 8-tile single-slot-8 calls (2)
WS = NMAX // 16              # 64 idx cols per single call
WG = NMAX // 16              # 64 idx cols per pair-group call

NQUEUES = 4

_CACHE = {}


def _build_program():
    nc = bacc.Bacc(
        "TRN2",
        target_bir_lowering=False,
        debug=False,
        num_devices=1,
        num_swdge_queues=NQUEUES,
    )

    xs = nc.dram_tensor("xs", [UMAX, E], BF16, kind="ExternalInput")
    idx = nc.dram_tensor("idx", [P, NS * WS + NG * WG], mybir.dt.int16,
                         kind="ExternalInput")
    out = nc.dram_tensor("out", [LPC, E], BF16, kind="ExternalOutput")

    xs_pairs = xs.ap().rearrange("(r f) e -> r (f e)", f=2)   # [9216, 2048]

    with TileContext(nc) as tc:
        with tc.tile_pool(name="sbuf", bufs=2) as pool:
            idx_sb = pool.tile([P, NS * WS + NG * WG], mybir.dt.int16, bufs=1)
            nc.sync.dma_start(out=idx_sb[:], in_=idx.ap())

            call_i = 0
            for q in range(NS):
                s8 = pool.tile([P, 8 * E], BF16, tag="s8")
                cq = q * (WS + 4 * WG)
                nc.gpsimd.dma_gather(
                    out_ap=s8[:].rearrange("p (g e) -> p g e", e=E),
                    in_ap=xs.ap(),
                    idxs_ap=idx_sb[:, cq : cq + WS],
                    num_idxs=8 * P,
                    num_idxs_reg=8 * P,
                    elem_size=E,
                    queue_num=call_i % NQUEUES,
                )
                call_i += 1
                for gi in range(4):
                    g = 4 * q + gi
                    gp = pool.tile([P, 8 * 2 * E], BF16, tag="gp", bufs=3)
                    c0 = q * (WS + 4 * WG) + WS + gi * WG
                    nc.gpsimd.dma_gather(
                        out_ap=gp[:].rearrange("p (g e) -> p g e", e=2 * E),
                        in_ap=xs_pairs,
                        idxs_ap=idx_sb[:, c0 : c0 + WG],
                        num_idxs=NMAX,
                        num_idxs_reg=NMAX,
                        elem_size=2 * E,
                        queue_num=call_i % NQUEUES,
                    )
                    call_i += 1
                    for ti in range(2):
                        t = 2 * g + ti
                        t4 = pool.tile([P, 4 * E], BF16, tag="t4")
                        for f in range(4):
                            blk = (2 * f + ti) * 2 * E
                            nc.vector.tensor_tensor(
                                out=t4[:, f * E : (f + 1) * E],
                                in0=gp[:, blk : blk + E],
                                in1=gp[:, blk + E : blk + 2 * E],
                                op=mybir.AluOpType.max,
                            )
                        t2 = pool.tile([P, 2 * E], BF16, tag="t2")
                        nc.vector.tensor_tensor(
                            out=t2[:], in0=t4[:, : 2 * E], in1=t4[:, 2 * E :],
                            op=mybir.AluOpType.max,
                        )
                        acc = pool.tile([P, E], BF16, tag="acc")
                        nc.vector.tensor_tensor(
                            out=acc[:], in0=t2[:, :E], in1=t2[:, E:],
                            op=mybir.AluOpType.max,
                        )
                        nc.vector.tensor_tensor(
                            out=acc[:], in0=acc[:],
                            in1=s8[:, (t % 8) * E : (t % 8 + 1) * E],
                            op=mybir.AluOpType.max,
                        )
                        nc.sync.dma_start(
                            out=out.ap()[t * P : (t + 1) * P, :], in_=acc[:]
                        )

    nc.compile()
    return nc


def _get_program():
    if "nc" not in _CACHE:
        _CACHE["nc"] = _build_program()
    return _CACHE["nc"]


def _wrap16(lst: np.ndarray) -> np.ndarray:
    """(N,) int -> (128, N/16) int16: 16-partition wrap, replicated x8."""
    w = len(lst) // 16
    return np.tile(lst.reshape(w, 16).T, (8, 1)).astype(np.int16)


def _host_prepare(x: np.ndarray, nb: np.ndarray) -> list[dict]:
    # (LIN, B*C) bf16: one 2KB row per input location
    xm = np.ascontiguousarray(x.transpose(2, 0, 1).reshape(LIN, E)).astype(
        ml_dtypes.bfloat16
    )

    in_maps = []
    for core in range(NCORE):
        sl = slice(core * LPC, (core + 1) * LPC)
        nbc = nb[:, sl]                                   # (K, LPC)
        xs = np.empty((UMAX, E), dtype=ml_dtypes.bfloat16)
        for f in range(NPAIR):
            # region f: rows f*4096 + 2*jl + s  =  xm[nbc[2f+s, jl]]
            xs[f * 2 * LPC : (f + 1) * 2 * LPC] = xm[
                nbc[2 * f : 2 * f + 2].T.ravel()
            ]
        xs[2 * NPAIR * LPC :] = xm[nbc[8]]

        jl = np.arange(NMAX)
        cols = []
        for q in range(NS):
            # slot-8 singles for tiles 8q..8q+7: entry i -> row 16384 + q*1024 + i
            cols.append(_wrap16(2 * NPAIR * LPC + q * NMAX + jl))
            for gi in range(4):
                g = 4 * q + gi
                # pair call for tiles 2g, 2g+1: entry i = f*256 + r ->
                # pair-row f*2048 + 2g*128 + r   (r in [0,256))
                f = jl // 256
                r = jl % 256
                cols.append(_wrap16(f * LPC + 2 * g * P + r))
        idx_np = np.ascontiguousarray(np.concatenate(cols, axis=1))
        in_maps.append({"xs": xs, "idx": idx_np})
    return in_maps


def kernel(x: np.ndarray, neighbours: np.ndarray) -> np.ndarray:
    x = np.asarray(x)
    nb = np.asarray(neighbours).astype(np.int64)          # (K, LOUT)
    assert x.shape == (B, C, LIN) and x.dtype == np.float32
    assert nb.shape == (K, LOUT)

    in_maps = _host_prepare(x, nb)
    nc = _get_program()
    res = bass_utils.run_bass_kernel_spmd(nc, in_maps, core_ids=list(range(NCORE)))
    _CACHE["last_result"] = res

    # out per core: (LPC, B*C) bf16 -> full (B, C, LOUT) f32
    dev = np.concatenate(
        [np.asarray(res.results[c]["out"]) for c in range(NCORE)]
    )  # (LOUT, E) bf16
    return np.ascontiguousarray(
        dev.reshape(LOUT, B, C).transpose(1, 2, 0)
    ).astype(np.float32)


# revision 21
# speedup vs baseline: 1.0074x; 1.0074x over previous
"""Trainium2 Bass kernel for nn_PoolNU: gather + max-pool over neighbour table.

reference:
    x: (8, 128, 65536) f32, neighbours: (9, 16384) int
    out[b, c, j] = max_k x[b, c, neighbours[k, j]]

Strategy:
    - The neighbour table is shared across (b, c), so one gathered "row"
      carries ALL batches and channels for a location: x repacked host-side to
      (65536, B*C=1024). Values are cast to bf16 (tolerance is 2e-2; bf16
      rounding is ~0.2%), halving all HBM traffic vs f32.
    - Output locations (16384) are sharded across the 8 NeuronCores (2048
      per core).
    - The bottleneck is SWDGE descriptor generation on the GpSimd Q7 cores
      (~9.4ns/index, serialized).  To cut descriptor count, the host packs
      the per-core table in PAIR rows: for output j, slots (2f, 2f+1) are
      stored as adjacent 2KB rows, so one 4KB descriptor fetches two
      neighbour values.  Slot 8 stays as single 2KB rows.  Per output:
      4 pair descriptors + 1 single = 10240 descriptors/core instead of
      18432.  (8KB descriptors crash the ucode; 4KB is the proven max.)
    - Device: per 2-tile group one 1024-index dma_gather covering all 4 pair
      regions; per 8 tiles one 1024-index gather for slot 8; vector max-tree
      in bf16; store 2KB rows. dma_gather calls round-robin over 4 SWDGE
      queues (pipelines descriptor gen a bit via instruction run-ahead).
    - Host reassembles (core, loc, b, c) -> (b, c, loc) and casts to f32.
"""

import sys

sys.path.insert(0, "/opt/trn_rl_repo")

import ml_dtypes
import numpy as np

import concourse.mybir as mybir
from concourse import bacc, bass_utils
from concourse.tile import TileContext

B = 8
C = 128
LIN = 65536
K = 9
LOUT = 16384

P = 128
NCORE = 8
LPC = LOUT // NCORE          # locations per core (2048)
NTILE = LPC // P             # tiles per core (16)
E = B * C                    # elements per gathered row (1024)
NPAIR = 4                    # pair regions (slots 0-7)
UMAX = LPC * (2 * NPAIR + 1)  # table rows (18432)
NMAX = 1024                  # max indices per dma_gather call

BF16 = mybir.dt.bfloat16

NS = NTILE // 8              # 8-tile single-slot-8 calls (2)
WS = NMAX // 16              # 64 idx cols per single call
WG = 512 // 16               # 32 idx cols per per-tile pair call

NQUEUES = 4

_CACHE = {}


def _build_program():
    nc = bacc.Bacc(
        "TRN2",
        target_bir_lowering=False,
        debug=False,
        num_devices=1,
        num_swdge_queues=NQUEUES,
    )

    xs = nc.dram_tensor("xs", [UMAX, E], BF16, kind="ExternalInput")
    idx = nc.dram_tensor("idx", [P, NS * WS + NTILE * WG], mybir.dt.int16,
                         kind="ExternalInput")
    out = nc.dram_tensor("out", [LPC, E], BF16, kind="ExternalOutput")

    xs_pairs = xs.ap().rearrange("(r f) e -> r (f e)", f=2)   # [9216, 2048]

    with TileContext(nc) as tc:
        with tc.tile_pool(name="sbuf", bufs=2) as pool:
            idx_sb = pool.tile([P, NS * WS + NTILE * WG], mybir.dt.int16, bufs=1)
            nc.sync.dma_start(out=idx_sb[:], in_=idx.ap())

            call_i = 0
            for q in range(NS):
                s8 = pool.tile([P, 8 * E], BF16, tag="s8")
                cq = q * (WS + 8 * WG)
                nc.gpsimd.dma_gather(
                    out_ap=s8[:].rearrange("p (g e) -> p g e", e=E),
                    in_ap=xs.ap(),
                    idxs_ap=idx_sb[:, cq : cq + WS],
                    num_idxs=8 * P,
                    num_idxs_reg=8 * P,
                    elem_size=E,
                    queue_num=call_i % NQUEUES,
                )
                call_i += 1
                for ti in range(8):
                    t = 8 * q + ti
                    gp = pool.tile([P, 4 * 2 * E], BF16, tag="gp", bufs=4)
                    c0 = q * (WS + 8 * WG) + WS + ti * WG
                    nc.gpsimd.dma_gather(
                        out_ap=gp[:].rearrange("p (g e) -> p g e", e=2 * E),
                        in_ap=xs_pairs,
                        idxs_ap=idx_sb[:, c0 : c0 + WG],
                        num_idxs=512,
                        num_idxs_reg=512,
                        elem_size=2 * E,
                        queue_num=call_i % NQUEUES,
                    )
                    call_i += 1
                    t4 = pool.tile([P, 4 * E], BF16, tag="t4")
                    for f in range(4):
                        blk = f * 2 * E
                        nc.vector.tensor_tensor(
                            out=t4[:, f * E : (f + 1) * E],
                            in0=gp[:, blk : blk + E],
                            in1=gp[:, blk + E : blk + 2 * E],
                            op=mybir.AluOpType.max,
                        )
                    t2 = pool.tile([P, 2 * E], BF16, tag="t2")
                    nc.vector.tensor_tensor(
                        out=t2[:], in0=t4[:, : 2 * E], in1=t4[:, 2 * E :],
                        op=mybir.AluOpType.max,
                    )
                    acc = pool.tile([P, E], BF16, tag="acc")
                    nc.vector.tensor_tensor(
                        out=acc[:], in0=t2[:, :E], in1=t2[:, E:],
                        op=mybir.AluOpType.max,
                    )
                    nc.vector.tensor_tensor(
                        out=acc[:], in0=acc[:],
                        in1=s8[:, ti * E : (ti + 1) * E],
                        op=mybir.AluOpType.max,
                    )
                    nc.sync.dma_start(
                        out=out.ap()[t * P : (t + 1) * P, :], in_=acc[:]
                    )

    nc.compile()
    return nc


def _get_program():
    if "nc" not in _CACHE:
        _CACHE["nc"] = _build_program()
    return _CACHE["nc"]


def _wrap16(lst: np.ndarray) -> np.ndarray:
    """(N,) int -> (128, N/16) int16: 16-partition wrap, replicated x8."""
    w = len(lst) // 16
    return np.tile(lst.reshape(w, 16).T, (8, 1)).astype(np.int16)


def _host_prepare(x: np.ndarray, nb: np.ndarray) -> list[dict]:
    # (LIN, B*C) bf16: one 2KB row per input location
    xm = np.ascontiguousarray(x.transpose(2, 0, 1).reshape(LIN, E)).astype(
        ml_dtypes.bfloat16
    )

    in_maps = []
    for core in range(NCORE):
        sl = slice(core * LPC, (core + 1) * LPC)
        nbc = nb[:, sl]                                   # (K, LPC)
        xs = np.empty((UMAX, E), dtype=ml_dtypes.bfloat16)
        for f in range(NPAIR):
            # region f: rows f*4096 + 2*jl + s  =  xm[nbc[2f+s, jl]]
            xs[f * 2 * LPC : (f + 1) * 2 * LPC] = xm[
                nbc[2 * f : 2 * f + 2].T.ravel()
            ]
        xs[2 * NPAIR * LPC :] = xm[nbc[8]]

        jl = np.arange(NMAX)
        jt = np.arange(512)
        cols = []
        for q in range(NS):
            # slot-8 singles for tiles 8q..8q+7: entry i -> row 16384 + q*1024 + i
            cols.append(_wrap16(2 * NPAIR * LPC + q * NMAX + jl))
            for ti in range(8):
                t = 8 * q + ti
                # pair call for tile t: entry i = f*128 + p ->
                # pair-row f*2048 + t*128 + p
                f = jt // P
                p = jt % P
                cols.append(_wrap16(f * LPC + t * P + p))
        idx_np = np.ascontiguousarray(np.concatenate(cols, axis=1))
        in_maps.append({"xs": xs, "idx": idx_np})
    return in_maps


def kernel(x: np.ndarray, neighbours: np.ndarray) -> np.ndarray:
    x = np.asarray(x)
    nb = np.asarray(neighbours).astype(np.int64)          # (K, LOUT)
    assert x.shape == (B, C, LIN) and x.dtype == np.float32
    assert nb.shape == (K, LOUT)

    in_maps = _host_prepare(x, nb)
    nc = _get_program()
    res = bass_utils.run_bass_kernel_spmd(nc, in_maps, core_ids=list(range(NCORE)))
    _CACHE["last_result"] = res

    # out per core: (LPC, B*C) bf16 -> full (B, C, LOUT) f32
    dev = np.concatenate(
        [np.asarray(res.results[c]["out"]) for c in range(NCORE)]
    )  # (LOUT, E) bf16
    return np.ascontiguousarray(
        dev.reshape(LOUT, B, C).transpose(1, 2, 0)
    ).astype(np.float32)


# revision 22
# speedup vs baseline: 1.1332x; 1.1248x over previous
"""Trainium2 Bass kernel for nn_PoolNU: gather + max-pool over neighbour table.

reference:
    x: (8, 128, 65536) f32, neighbours: (9, 16384) int
    out[b, c, j] = max_k x[b, c, neighbours[k, j]]

Strategy:
    - The neighbour table is shared across (b, c), so one gathered "row"
      carries ALL batches and channels for a location: x repacked host-side to
      (65536, B*C=1024). Values are cast to bf16 (tolerance is 2e-2; bf16
      rounding is ~0.2%), halving all HBM traffic: 2KB rows instead of 4KB.
    - Output locations (16384) are sharded across the 8 NeuronCores (2048
      per core). Each core needs at most 9*2048=18432 distinct source rows,
      which the host compacts into a per-core x_sub with remapped indices —
      guaranteed to fit dma_gather's int16 index window (< 32768).
    - dma_gather is limited to 1024 indices per call (64-descriptor packet
      ceiling per SDMA lane with single_packet). Device per tile of 128
      locations: gather slots 0-7 (1024 idxs), slot 8 gathered per quarter
      (512 idxs), vector max-tree in bf16, store 2KB rows.
    - Host reassembles (core, loc, b, c) -> (b, c, loc) and casts to f32.
"""

import sys

sys.path.insert(0, "/opt/trn_rl_repo")

import ml_dtypes
import numpy as np

import concourse.mybir as mybir
from concourse import bacc, bass_utils
from concourse.tile import TileContext

B = 8
C = 128
LIN = 65536
K = 9
LOUT = 16384

P = 128
NCORE = 8
LPC = LOUT // NCORE          # locations per core (2048)
NTILE = LPC // P             # tiles per core (16)
E = B * C                    # elements per gathered row (1024)
UMAX = K * LPC               # padded x_sub rows (18432)
NMAX = 1024                  # max indices per dma_gather call

BF16 = mybir.dt.bfloat16

WA = NMAX // 16              # 64 idx cols per tile call
WQ = 8 * P // 16             # 64 idx cols per half slot-8 call (1024 idxs)
NQ = NTILE // 8              # two slot-8 calls, each covering 8 tiles

_CACHE = {}


NQUEUES = 4


def _build_program():
    nc = bacc.Bacc(
        "TRN2",
        target_bir_lowering=False,
        debug=False,
        num_devices=1,
        num_swdge_queues=NQUEUES,
    )

    xs = nc.dram_tensor("xs", [UMAX, E], BF16, kind="ExternalInput")
    # idx layout per core: per tile one 1024-index call (slots 0..7), then per
    # quarter (4 tiles) one 512-index call for slot 8. All 16-wrapped and
    # replicated over the 128 partitions in groups of 16.
    idx = nc.dram_tensor("idx", [P, NTILE * WA + NQ * WQ], mybir.dt.int16,
                         kind="ExternalInput")
    out = nc.dram_tensor("out", [LPC, E], BF16, kind="ExternalOutput")

    with TileContext(nc) as tc:
        with tc.tile_pool(name="sbuf", bufs=2) as pool:
            idx_sb = pool.tile([P, NTILE * WA + NQ * WQ], mybir.dt.int16, bufs=1)
            nc.sync.dma_start(out=idx_sb[:], in_=idx.ap())

            call_i = 0
            for q in range(NQ):
                s8 = pool.tile([P, 8 * E], BF16, tag="s8")
                cq = NTILE * WA + q * WQ
                nc.gpsimd.dma_gather(
                    out_ap=s8[:].rearrange("p (g e) -> p g e", e=E),
                    in_ap=xs.ap(),
                    idxs_ap=idx_sb[:, cq : cq + WQ],
                    num_idxs=8 * P,
                    num_idxs_reg=8 * P,
                    elem_size=E,
                    queue_num=call_i % NQUEUES,
                )
                call_i += 1
                for ti in range(8):
                    t = q * 8 + ti
                    g = pool.tile([P, 8 * E], BF16, tag="g", bufs=3)
                    c0 = t * WA
                    nc.gpsimd.dma_gather(
                        out_ap=g[:].rearrange("p (g e) -> p g e", e=E),
                        in_ap=xs.ap(),
                        idxs_ap=idx_sb[:, c0 : c0 + WA],
                        num_idxs=NMAX,
                        num_idxs_reg=NMAX,
                        elem_size=E,
                        queue_num=call_i % NQUEUES,
                    )
                    call_i += 1
                    t4 = pool.tile([P, 4 * E], BF16, tag="t4")
                    nc.vector.tensor_tensor(
                        out=t4[:], in0=g[:, : 4 * E], in1=g[:, 4 * E :],
                        op=mybir.AluOpType.max,
                    )
                    t2 = pool.tile([P, 2 * E], BF16, tag="t2")
                    nc.vector.tensor_tensor(
                        out=t2[:], in0=t4[:, : 2 * E], in1=t4[:, 2 * E :],
                        op=mybir.AluOpType.max,
                    )
                    acc = pool.tile([P, E], BF16, tag="acc")
                    nc.vector.tensor_tensor(
                        out=acc[:], in0=t2[:, :E], in1=t2[:, E:],
                        op=mybir.AluOpType.max,
                    )
                    nc.vector.tensor_tensor(
                        out=acc[:], in0=acc[:], in1=s8[:, ti * E : (ti + 1) * E],
                        op=mybir.AluOpType.max,
                    )
                    nc.sync.dma_start(
                        out=out.ap()[t * P : (t + 1) * P, :], in_=acc[:]
                    )

    nc.compile()
    return nc


def _get_program():
    if "nc" not in _CACHE:
        _CACHE["nc"] = _build_program()
    return _CACHE["nc"]


def _wrap16(lst: np.ndarray) -> np.ndarray:
    """(N,) int -> (128, N/16) int16: 16-partition wrap, replicated x8."""
    w = len(lst) // 16
    return np.tile(lst.reshape(w, 16).T, (8, 1)).astype(np.int16)


def _host_prepare(x: np.ndarray, nb: np.ndarray) -> list[dict]:
    # (LIN, B*C) bf16: one 2KB row per input location
    xm = np.ascontiguousarray(x.transpose(2, 0, 1).reshape(LIN, E)).astype(
        ml_dtypes.bfloat16
    )

    in_maps = []
    for core in range(NCORE):
        nbc = nb[:, core * LPC : (core + 1) * LPC]        # (K, LPC)
        uniq, inv = np.unique(nbc, return_inverse=True)
        inv = inv.reshape(K, LPC)
        xs = np.zeros((UMAX, E), dtype=ml_dtypes.bfloat16)
        xs[: len(uniq)] = xm[uniq]
        cols = []
        for t in range(NTILE):
            loc2d = inv[:, t * P : (t + 1) * P]           # (K, P) local idx
            # per-tile call: slots 0..7 -> list[s*128+p] = loc2d[s, p]
            cols.append(_wrap16(loc2d[:8].ravel()))
        for q in range(NQ):
            # per-half slot-8 call: list[g*128+p] = inv[8, (q*8+g)*P + p]
            cols.append(_wrap16(inv[8, q * 8 * P : (q + 1) * 8 * P]))
        idx_np = np.ascontiguousarray(np.concatenate(cols, axis=1))
        in_maps.append({"xs": xs, "idx": idx_np})
    return in_maps


def kernel(x: np.ndarray, neighbours: np.ndarray) -> np.ndarray:
    x = np.asarray(x)
    nb = np.asarray(neighbours).astype(np.int64)          # (K, LOUT)
    assert x.shape == (B, C, LIN) and x.dtype == np.float32
    assert nb.shape == (K, LOUT)

    in_maps = _host_prepare(x, nb)
    nc = _get_program()
    res = bass_utils.run_bass_kernel_spmd(nc, in_maps, core_ids=list(range(NCORE)))
    _CACHE["last_result"] = res

    # out per core: (LPC, B*C) bf16 -> full (B, C, LOUT) f32
    dev = np.concatenate(
        [np.asarray(res.results[c]["out"]) for c in range(NCORE)]
    )  # (LOUT, E) bf16
    return np.ascontiguousarray(
        dev.reshape(LOUT, B, C).transpose(1, 2, 0)
    ).astype(np.float32)
